# revision 79
# baseline (speedup 1.0000x reference)
"""Trainium2 Bass kernel for a dense transformer block (PreNorm attn + PreNorm MLP).

Sharding (8 cores, collective-free): core c -> batch b = c//2, sequence half
h = c%2.  Each core computes K/V for the full 2048-token sequence of its batch
element (redundant across the core pair) but Q/attention/FFN only for its own
1024 tokens.  The host permutes each core's token axis so the core's OWN 1024
tokens always occupy columns 0:1024 (softmax is invariant to key order), which
lets one compiled program serve all 8 cores with z_q a plain slice of z_full.

Layout: activations are feature-major ([feature, token]).  Weights are
host-pre-tiled so every DMA is one contiguous block; LN affines are folded into
downstream weights; LN stats come from ones-vector matmuls.

Attention is restructured around the cost model:
  * scores st[key, query] (f32r QK, 512-wide moving) -> exp on ACT over a
    2-head [128, 1024] PSUM tile -> AV with pexp as the STATIONARY operand:
    out[query, dv] = sum_k pexp[k, q] v[k, d].  This halves tensor-engine AV
    time vs the [dv+1, q] orientation (moving dim 65 vs 512 per key tile)
    and gives token-major AV output that is transposed back per 128x128
    block on the PE (free: Ldweights costs nothing, transpose 1cyc/row bf16).
  * the softmax denominator rides as an appended ones column of V (col 64),
    the per-query shift M rides as contraction row 64 (ones in kaug, -M in qt).
  * queries are processed in two 512-column segments over all 12 heads; the
    out-projection + LN2 + FFN of segment 0 overlaps segment 1's ACT-bound
    exp work, keeping the PE busy.  K (f32r kaug) is bounced through DRAM
    between segments instead of recomputed.

Precision: score path (z, wq/wk, q, k, QK) in float32r; V/out/FFN bf16.
"""

import sys

sys.path.insert(0, "/opt/trn_rl_repo")

import numpy as np

import concourse.bacc as bacc
import concourse.bass as bass
import concourse.tile as tile
from concourse import mybir
from concourse.bass_utils import run_bass_kernel_spmd

F32 = mybir.dt.float32
F32R = mybir.dt.float32r
BF16 = mybir.dt.bfloat16
AF = mybir.ActivationFunctionType
ALU = mybir.AluOpType

D = 768
H = 12
HP = 6  # head pairs
DH = 64
F = 3072
B = 4
N = 2048
NQ = 1024  # tokens owned per core
P = 128
KT = D // P  # 6 feature k-tiles
MT = F // P  # 24 mlp-hidden tiles
NKT = N // P  # 16 key-token tiles
SG = 512  # query segment width
NSEG = NQ // SG  # 2
QB = SG // P  # 4 query blocks per segment
SCALE = float(DH**0.5)  # reference MULTIPLIES scores by sqrt(dh)
EXP_BIAS = -40.0  # pad on the exp argument (post-scale logit units)
SSTRIDE = 16  # key sampling stride for the shift estimate
NS = N // SSTRIDE
EPS = 1e-5
CK = 512


def build_nc():
    nc = bacc.Bacc("TRN2", target_bir_lowering=False, debug=False)

    xT = nc.dram_tensor("xT", [N // CK, P, KT, CK], F32R, kind="ExternalInput")
    xTq2 = nc.dram_tensor("xTq2", [D, NQ], F32, kind="ExternalInput")
    wq = nc.dram_tensor("wq", [KT, P, KT, P], F32R, kind="ExternalInput")
    wk = nc.dram_tensor("wk", [KT, P, KT, P], F32R, kind="ExternalInput")
    wv = nc.dram_tensor("wv", [2, P, KT, CK], F32R, kind="ExternalInput")
    wo = nc.dram_tensor("wo", [KT, P, KT, P], BF16, kind="ExternalInput")
    w1 = nc.dram_tensor("w1", [MT, P, KT, P], BF16, kind="ExternalInput")
    w2 = nc.dram_tensor("w2", [KT, P, MT, P], BF16, kind="ExternalInput")
    bq = nc.dram_tensor("bq", [D], F32, kind="ExternalInput")
    bv = nc.dram_tensor("bv", [D], F32R, kind="ExternalInput")
    bo = nc.dram_tensor("bo", [D], F32, kind="ExternalInput")
    b1 = nc.dram_tensor("b1", [F], F32, kind="ExternalInput")
    b2 = nc.dram_tensor("b2", [D], F32, kind="ExternalInput")
    ident = nc.dram_tensor("ident", [P, P], BF16, kind="ExternalInput")
    onesd = nc.dram_tensor("onesd", [1, N], F32R, kind="ExternalInput")
    yT = nc.dram_tensor("yT", [D, NQ], F32, kind="ExternalOutput")
    mscratch = nc.dram_tensor("mscratch", [H, NQ], F32R)
    qdram = nc.dram_tensor("qdram", [H, DH, SG], F32R)
    kdram = nc.dram_tensor("kdram", [H, DH + 1, N], F32R)

    with tile.TileContext(nc) as tc:
        _body(tc, xT, xTq2, wq, wk, wv, wo, w1, w2, bq, bv, bo, b1, b2,
              ident, onesd, yT, mscratch, qdram, kdram)
    nc.compile()
    return nc


class Ctx:
    pass


def _layernorm_fm(tc, g, load_fn, ncols, name, zpool, out_dt, wp, rstd_on_act=False,
                  lnps=None, on_chunk=None, z_tiles=None, rb=1):
    """Feature-major layernorm (affine folded into downstream weights).

    load_fn(k, c, sl) -> AP of a [128, CK] chunk of the input.
    Returns KT tiles [128, ncols] of dtype out_dt holding z = (x - mu) * rstd.
    """
    nc = tc.nc
    nch = ncols // CK
    ones_row = g.ones_row_r if out_dt == F32R else g.ones_row_b
    row_dt = F32R if out_dt == F32R else BF16

    z_sb = z_tiles if z_tiles is not None else [
        zpool.tile([P, ncols], out_dt, name=f"{name}_z{k}") for k in range(KT)]
    for c in range(nch):
        sl = slice(c * CK, (c + 1) * CK)
        if lnps is not None:
            ps = lnps.tile([33, CK], F32, tag="lnst", bufs=2, name="lnst_ps")
        else:
            ps = g.psum_mm.tile([33, CK], F32, tag="mm", name="ln_ps")
        p1, p2 = ps[0:1, :], ps[32:33, :]
        for k in range(KT):
            xc = load_fn(k, c, sl)
            if xc.dtype == F32R:
                nc.tensor.matmul(p1[:], g.ones_col_r[:], xc,
                                 start=(k == 0), stop=(k == KT - 1))
            elif xc.dtype == BF16:
                nc.tensor.matmul(p1[:], g.ones_col[:], xc,
                                 start=(k == 0), stop=(k == KT - 1))
            else:
                xb = wp.tile([P, CK], BF16, tag="ln_xb")
                nc.vector.tensor_copy(out=xb[:], in_=xc)
                nc.tensor.matmul(p1[:], g.ones_col[:], xb[:],
                                 start=(k == 0), stop=(k == KT - 1))
            xsq = wp.tile([P, CK], BF16, tag="ln_xsq", bufs=2)
            nc.scalar.activation(out=xsq[:], in_=xc, func=AF.Square)
            nc.tensor.matmul(p2[:], g.ones_col[:], xsq[:], start=(k == 0), stop=(k == KT - 1))
        s1 = wp.tile([1, CK], F32, name="s1r", tag="lnr_a", bufs=rb)
        s2 = wp.tile([1, CK], F32, name="s2r", tag="lnr_b", bufs=rb)
        rt = wp.tile([1, CK], F32, name="rtr", tag="lnr_c", bufs=1)
        y = wp.tile([1, CK], F32, name="yr", tag="lnr_y", bufs=1)
        nc.vector.tensor_scalar_mul(out=s1[:], in0=p1[:], scalar1=1.0 / D)
        nc.vector.tensor_scalar(out=s2[:], in0=p2[:], scalar1=1.0 / D,
                                scalar2=EPS, op0=ALU.mult, op1=ALU.add)
        nc.vector.tensor_mul(out=rt[:], in0=s1[:], in1=s1[:])  # mu^2
        nc.vector.tensor_sub(out=s2[:], in0=s2[:], in1=rt[:])  # var + eps
        if rstd_on_act:
            # rstd = exp(-0.5*ln(var)): fine where ACT is idle (preamble);
            # costs two act-table loads per chunk
            nc.scalar.activation(out=rt[:], in_=s2[:], func=AF.Ln)
            nc.scalar.activation(out=y[:], in_=rt[:], func=AF.Exp, scale=-0.5)
        else:
            # rstd = rsqrt(var) via Newton on DVE (vars are ~1, so a linear
            # seed converges in 3 steps); keeps LN2 off the ACT tables while
            # the attention exp stream runs
            nc.vector.tensor_scalar(out=y[:], in0=s2[:], scalar1=-0.5,
                                    scalar2=1.5, op0=ALU.mult, op1=ALU.add)
            for _ in range(3):
                nc.vector.tensor_mul(out=rt[:], in0=y[:], in1=y[:])  # y^2
                nc.vector.tensor_mul(out=rt[:], in0=rt[:], in1=s2[:])  # v*y^2
                nc.vector.tensor_scalar(out=rt[:], in0=rt[:], scalar1=-0.5,
                                        scalar2=1.5, op0=ALU.mult, op1=ALU.add)
                nc.vector.tensor_mul(out=y[:], in0=y[:], in1=rt[:])
        nc.vector.tensor_mul(out=s1[:], in0=s1[:], in1=y[:])  # m2 = mu*rstd
        rstd_r = wp.tile([1, CK], row_dt, name="rstdr", tag="lnr_d", bufs=2)
        nc.vector.tensor_copy(out=rstd_r[:], in_=y[:])
        m2_r = wp.tile([1, CK], row_dt, name="m2r", tag="lnr_e", bufs=2)
        nc.vector.tensor_copy(out=m2_r[:], in_=s1[:])

        if lnps is not None:
            rstdF = lnps.tile([P, CK], F32, tag="lnbc", bufs=4, name="rstdF")
            m2F = lnps.tile([P, CK], F32, tag="lnbc", bufs=4, name="m2F")
        else:
            rstdF = g.psum_mm.tile([P, CK], F32, tag="mm")
            m2F = g.psum_mm.tile([P, CK], F32, tag="mm")
        nc.tensor.matmul(rstdF[:], ones_row[:], rstd_r[:], start=True, stop=True)
        nc.tensor.matmul(m2F[:], ones_row[:], m2_r[:], start=True, stop=True)
        m2F_sb = wp.tile([P, CK], F32, tag="m2fsb", bufs=2)
        nc.scalar.copy(out=m2F_sb[:], in_=m2F[:])
        for k in range(KT):
            xc = load_fn(k, c, sl)
            tmp = wp.tile([P, CK], F32, tag="lntmp", bufs=2)
            nc.vector.tensor_mul(out=tmp[:], in0=xc, in1=rstdF[:])
            nc.gpsimd.tensor_sub(out=z_sb[k][:, sl], in0=tmp[:], in1=m2F_sb[:])
        if on_chunk is not None:
            on_chunk(c)
    return z_sb


def _body(tc, xT, xTq2, wq, wk, wv, wo, w1, w2, bq, bv, bo, b1, b2,
          ident, onesd, yT, mscratch, qdram, kdram):
    nc = tc.nc
    from contextlib import ExitStack

    with ExitStack() as es:
        g = Ctx()
        g.singles = es.enter_context(tc.tile_pool(name="singles", bufs=1))
        g.rows = es.enter_context(tc.tile_pool(name="rows", bufs=1))
        g.work = es.enter_context(tc.tile_pool(name="work", bufs=2))
        g.wpool = es.enter_context(tc.tile_pool(name="wpool", bufs=2))
        # PSUM: mm [128,512] x2 up front; st/av created after the LN phase
        # (LN1 borrows their banks for chunk pipelining)
        g.psum_mm = es.enter_context(tc.tile_pool(name="psum_mm", bufs=2, space="PSUM"))

        g.ones_col = g.singles.tile([P, 1], BF16, name="ones_col")
        nc.vector.memset(g.ones_col[:], 1.0)
        g.ones_col_f = g.singles.tile([P, 1], F32, name="ones_col_f")
        nc.vector.memset(g.ones_col_f[:], 1.0)
        g.ones_col_r = g.singles.tile([P, 1], F32R, name="ones_col_r")
        nc.vector.tensor_copy(out=g.ones_col_r[:], in_=g.ones_col_f[:])
        g.ones_row_b = g.singles.tile([1, P], BF16, name="ones_row_b")
        nc.vector.memset(g.ones_row_b[:], 1.0)
        g.ones_row_f = g.singles.tile([1, P], F32, name="ones_row_f")
        nc.vector.memset(g.ones_row_f[:], 1.0)
        g.ones_row_r = g.singles.tile([1, P], F32R, name="ones_row_r")
        nc.vector.tensor_copy(out=g.ones_row_r[:], in_=g.ones_row_f[:])
        g.eps_sb = g.singles.tile([1, 1], F32, name="eps")
        nc.vector.memset(g.eps_sb[:], EPS)
        g.expb_sb = g.singles.tile([P, 1], F32, name="expb")
        nc.vector.memset(g.expb_sb[:], EXP_BIAS)
        g.zeros_row = g.singles.tile([1, QB * (DH + 1)], BF16, name="zeros_row")
        nc.vector.memset(g.zeros_row[:], 0.0)
        g.gelu_gate = g.singles.tile([P, 1], F32, name="gelu_gate")
        g.ident = g.singles.tile([P, P], BF16, name="ident")
        nc.sync.dma_start(out=g.ident[:], in_=ident.ap())

        def load_bias_cols(dram, n, name):
            t = g.singles.tile([P, n // P], F32, name=name)
            nc.sync.dma_start(out=t[:], in_=dram.ap().rearrange("(j p) -> p j", p=P))
            return t

        bo_sb = load_bias_cols(bo, D, "bo_sb")
        b1_sb = load_bias_cols(b1, F, "b1_sb")
        b2_sb = load_bias_cols(b2, D, "b2_sb")
        bq_sb = load_bias_cols(bq, D, "bq_sb")
        bv_row = g.singles.tile([1, D], F32R, name="bv_row")
        nc.sync.dma_start(out=bv_row[:], in_=bv.ap().rearrange("(a n) -> a n", a=1))

        def stream_loader(dram, pool):
            state = {}
            def load(k, c, sl):
                if state.get("c") != c:
                    t = pool.tile([P, KT, CK], F32R, tag="xstream", name="xs")
                    nc.sync.dma_start(out=t[:], in_=dram.ap()[c])
                    state["t"] = t
                    state["c"] = c
                return state["t"][:, k, :]
            return load

        # ---------- persistent activation tiles ----------
        vpool = es.enter_context(tc.tile_pool(name="vpool", bufs=1))
        v_sb = [vpool.tile([P, H, DH + 1], BF16, name=f"v{t}") for t in range(NKT)]
        opool = es.enter_context(tc.tile_pool(name="opool", bufs=1))
        o_sb = [[opool.tile([P, SG], BF16, name=f"o0_{j}") for j in range(KT)], None]
        xmid = [None, None]  # filled per segment from scoped pools

        for t in range(NKT):
            nc.gpsimd.memset(v_sb[t][:], 1.0)  # col 64 of each head stays 1.0

        def w_load(dram, j, tag, dt=BF16, nk=KT, w=P, pool=None, bufs=None):
            t = (pool or g.wpool).tile([P, nk, w], dt, tag=tag, name=f"wt_{tag}{j}",
                                       **({"bufs": bufs} if bufs else {}))
            nc.sync.dma_start(out=t[:], in_=dram.ap()[j])
            return t


        def q_proj_pair(jp, z, cols, out_even, out_odd, spool, on_act=False):
            """Project the head pair jp's queries for z[:, cols].

            Both heads come out of one [128, SG] psum (full PE width).  The
            odd head's rows 64:128 are biased into a staging tile and DMA'd
            to out_odd (partition shift needs a DMA).  With on_act the bias
            adds run as ACT Identity-with-bias (keeps the preamble DVE queue
            clear) and the even head is written directly to its SBUF AP."""
            wqb = w_load(wq, jp, "wqk", dt=F32R, pool=wqkp)
            pt = g.psum_mm.tile([P, SG], F32, tag="mm")
            for k in range(KT):
                nc.tensor.matmul(pt[:], wqb[:, k, :], z[k][:, cols],
                                 start=(k == 0), stop=(k == KT - 1))
            stg = spool.tile([P, SG], F32R, tag="qstg")
            if on_act:
                nc.scalar.activation(out=out_even, in_=pt[0:DH, :],
                                     func=AF.Identity,
                                     bias=bq_sb[0:DH, jp : jp + 1], scale=1.0)
                nc.scalar.activation(out=stg[DH:P, :], in_=pt[DH:P, :],
                                     func=AF.Identity,
                                     bias=bq_sb[DH:P, jp : jp + 1], scale=1.0)
            else:
                nc.vector.tensor_scalar_add(out=stg[0:DH, :], in0=pt[0:DH, :],
                                            scalar1=bq_sb[0:DH, jp : jp + 1])
                nc.vector.tensor_scalar_add(out=stg[DH:P, :], in0=pt[DH:P, :],
                                            scalar1=bq_sb[DH:P, jp : jp + 1])
                nc.sync.dma_start(out=out_even, in_=stg[0:DH, :])
            nc.sync.dma_start(out=out_odd, in_=stg[DH:P, :])

        def m_shift(h, q_sb, seg):
            """Sampled row-max shift for head h, queries of segment seg.

            q_sb rows 0:64 hold the biased q.  Writes -max to mscratch[h, seg]."""
            kaug = kaugs[h]
            ksamp = kaug[0:DH, :].rearrange("p (n t) -> p n t", t=SSTRIDE)[:, :, 0:1]
            m_sb = g.work.tile([P, QB], F32R, tag="msb")
            for qt_i in range(QB):
                ss = g.psum_mm.tile([P, CK], F32, tag="mm")
                nc.tensor.matmul(ss[:, :NS], q_sb[0:DH, qt_i * P : (qt_i + 1) * P],
                                 ksamp, start=True, stop=True)
                nc.vector.tensor_reduce(
                    out=m_sb[:, qt_i : qt_i + 1], in_=ss[:, :NS],
                    axis=mybir.AxisListType.X, op=ALU.max, negate=True,
                )
            nc.sync.dma_start(
                out=mscratch.ap()[h : h + 1, seg * SG : (seg + 1) * SG]
                    .rearrange("a (c p) -> a p c", p=P),
                in_=m_sb[:],
            )

        def k_proj(jp, kpool):
            """K projection for head pair jp -> kaug tiles in SBUF + kdram.

            One [128, CK] psum per chunk covers both heads (full PE width);
            the odd head's rows 64:128 bounce through a staging tile + DMA."""
            ks = []
            for s in range(2):
                h = 2 * jp + s
                kaug = kpool.tile([DH + 1, N], F32R, name=f"kaug{h}", tag="kaug")
                nc.sync.dma_start(out=kaug[DH : DH + 1, :], in_=onesd.ap())
                ks.append(kaug)
            wkb = w_load(wk, jp, "wqk", dt=F32R, pool=wqkp)
            for c in range(N // CK):
                sl = slice(c * CK, (c + 1) * CK)
                pt = g.psum_mm.tile([P, CK], F32, tag="mm")
                for k in range(KT):
                    nc.tensor.matmul(pt[:], wkb[:, k, :], z_full[k][:, sl],
                                     start=(k == 0), stop=(k == KT - 1))
                nc.vector.tensor_copy(out=ks[0][0:DH, sl], in_=pt[0:DH, :])
                stg = qspool.tile([P, CK], F32R, tag="kstg", bufs=2)
                nc.vector.tensor_copy(out=stg[DH:P, :], in_=pt[DH:P, :])
                nc.sync.dma_start(out=ks[1][0:DH, sl], in_=stg[DH:P, :])
            for s in range(2):
                nc.sync.dma_start(out=kdram.ap()[2 * jp + s], in_=ks[s][:])
            return ks

        def attn_pair(jp, seg, fillers, pexpool):
            """Attention for head pair jp over segment seg's 512 queries.

            fillers: list of zero-arg callables emitting independent PE work,
            interleaved into the t-loop to cover ACT-bound stretches."""
            h0, h1 = 2 * jp, 2 * jp + 1
            qts = (qt0[h0], qt0[h1]) if seg == 0 else (qt1s[h0], qt1s[h1])
            av = [g.psum_av.tile([P, QB * (DH + 1)], F32, tag="av", name=f"av{s}")
                  for s in range(2)]
            # The 4 query-block accumulation regions share one PSUM zero
            # region (2KB bank), so start_tensor_calc must fire exactly once
            # per bank: zero the whole tile with one K=1 matmul, then
            # accumulate with start=False.
            for s in range(2):
                nc.tensor.matmul(av[s][:], g.ones_row_b[:], g.zeros_row[:],
                                 start=True, stop=True)
            nfill = len(fillers)
            fi = 0
            for t in range(NKT):
                st = g.psum_st.tile([P, 2 * SG], F32, tag="st")
                for s in range(2):
                    nc.tensor.matmul(st[:, s * SG : (s + 1) * SG],
                                     kaugs[2 * jp + s][:, t * P : (t + 1) * P],
                                     qts[s][:], start=True, stop=True)
                pexp = pexpool.tile([P, 2 * SG], BF16, tag="pexp")
                nc.scalar.activation(out=pexp[:], in_=st[:], func=AF.Exp,
                                     scale=SCALE, bias=g.expb_sb[:])
                for s in range(2):
                    h = 2 * jp + s
                    for qb in range(QB):
                        nc.tensor.matmul(
                            av[s][:, qb * (DH + 1) : (qb + 1) * (DH + 1)],
                            pexp[:, s * SG + qb * P : s * SG + (qb + 1) * P],
                            v_sb[t][:, h, :],
                            start=False, stop=(t == NKT - 1),
                            skip_group_check=True)
                # interleave filler work so the PE queue stays fed while
                # exp(t+1) is still on ACT
                while fi * NKT < (t + 1) * nfill:
                    fillers[fi]()
                    fi += 1
            # normalize (token-major), then transpose pairs back to
            # feature-major o_sb via PE
            otok = g.work.tile([P, QB, P], BF16, tag="otok")
            rr = [g.work.tile([P, QB, 1], F32, tag="attn_r", bufs=4, name=f"r{s}")
                  for s in range(2)]
            for s in range(2):
                nc.vector.reciprocal(
                    out=rr[s][:],
                    in_=av[s][:].rearrange("p (q c) -> p q c", c=DH + 1)[:, :, DH : DH + 1])
            for qb in range(QB):
                for s in range(2):
                    nc.vector.tensor_scalar_mul(
                        out=otok[:, qb, s * DH : (s + 1) * DH],
                        in0=av[s][:, qb * (DH + 1) : qb * (DH + 1) + DH],
                        scalar1=rr[s][:, qb, :])
            for qb in range(QB):
                ptr = g.psum_av.tile([P, P], BF16, tag="av", name="ptr")
                nc.tensor.transpose(ptr[:], otok[:, qb, :], g.ident[:])
                nc.vector.tensor_copy(out=o_sb[seg][jp][:, qb * P : (qb + 1) * P],
                                      in_=ptr[:])

        def out_proj_fills(seg, wfp):
            """Closures: o_sb[seg] @ wo + bo + x residual -> xmid[seg]."""
            xq2 = [wfp.tile([P, SG], F32, tag="xq2", bufs=6, name=f"xq2_{seg}_{k}")
                   for k in range(KT)]

            def xq2_load():
                for k in range(KT):
                    nc.sync.dma_start(
                        out=xq2[k][:],
                        in_=xTq2.ap()[k * P : (k + 1) * P, seg * SG : (seg + 1) * SG])

            def op_j(j):
                pt = g.psum_mm.tile([P, SG], F32, tag="mm")
                for k in range(KT):
                    nc.tensor.matmul(pt[:], wobs[j][:, k, :], o_sb[seg][k][:],
                                     start=(k == 0), stop=(k == KT - 1))
                tmp = wfp.tile([P, SG], F32, tag="tmpf4")
                nc.vector.tensor_scalar_add(out=tmp[:], in0=pt[:],
                                            scalar1=bo_sb[:, j : j + 1])
                nc.vector.tensor_add(out=xmid[seg][j][:], in0=tmp[:], in1=xq2[j][:])

            return [xq2_load] + [lambda j=j: op_j(j) for j in range(KT)]

        def ffn_fills(seg, z2pool, h1pool, wfp, defer_gelu, rstd_on_act=False):
            """Closure groups for LN2 + FFN over segment seg's tokens.

            With defer_gelu, ffn1 stores biased pre-gelu h1 via DVE (so no
            Gelu touches ACT while the attention exp stream is running) and
            the returned gelu fills apply Gelu in place later.  Returns
            (pre_fills, gelu_fills, ffn2_fills)."""
            z2 = []
            h1 = [h1pool.tile([P, SG], BF16, name=f"h1_{seg}_{m}") for m in range(MT)]

            def ln2():
                z2.extend(_layernorm_fm(tc, g, lambda k, c, sl: xmid[seg][k][:, sl],
                                        SG, f"ln2_{seg}", z2pool, BF16, wp=wfp,
                                        rstd_on_act=rstd_on_act))

            def ffn1(m):
                w1b = w_load(w1, m, "w1", pool=wfp, bufs=3)
                pt = g.psum_mm.tile([P, SG], F32, tag="mm")
                for k in range(KT):
                    nc.tensor.matmul(pt[:], w1b[:, k, :], z2[k][:],
                                     start=(k == 0), stop=(k == KT - 1))
                if defer_gelu:
                    nc.vector.tensor_scalar_add(out=h1[m][:], in0=pt[:],
                                                scalar1=b1_sb[:, m : m + 1])
                else:
                    nc.scalar.activation(out=h1[m][:], in_=pt[:], func=AF.Gelu,
                                         bias=b1_sb[:, m : m + 1], scale=1.0)

            def gelu(m):
                # bias is numerically zero but reads the gate tile, which is
                # written only after the last attention output lands: all
                # gelus become ready together, after the exp stream, so the
                # act-table pass switches tables once instead of thrashing
                nc.scalar.activation(out=h1[m][:], in_=h1[m][:], func=AF.Gelu,
                                     bias=g.gelu_gate[:], scale=1.0)

            def ffn2(j):
                w2b = w_load(w2, j, "w2", nk=MT, pool=wfp, bufs=2)
                pt = g.psum_mm.tile([P, SG], F32, tag="mm")
                for k in range(MT):
                    nc.tensor.matmul(pt[:], w2b[:, k, :], h1[k][:],
                                     start=(k == 0), stop=(k == MT - 1))
                tmp = wfp.tile([P, SG], F32, tag="tmpf")
                nc.vector.tensor_scalar_add(out=tmp[:], in0=pt[:],
                                            scalar1=b2_sb[:, j : j + 1])
                out_t = wfp.tile([P, SG], F32, tag="f2_out")
                nc.vector.tensor_add(out=out_t[:], in0=tmp[:], in1=xmid[seg][j][:])
                nc.sync.dma_start(
                    out=yT.ap()[j * P : (j + 1) * P, seg * SG : (seg + 1) * SG],
                    in_=out_t[:])

            pre = [ln2] + [lambda m=m: ffn1(m) for m in range(MT)]
            gelus = [] if not defer_gelu else [lambda m=m: gelu(m) for m in range(MT)]
            return pre, gelus, [lambda j=j: ffn2(j) for j in range(KT)]

        # ================= preamble + segment 0 =================
        kaugs = {}
        with ExitStack() as pre_es:
            zpool = pre_es.enter_context(tc.tile_pool(name="z", bufs=1))
            z_full = [zpool.tile([P, N], F32R, name=f"ln1_z{k}") for k in range(KT)]
            qpool0 = pre_es.enter_context(tc.tile_pool(name="qpool0", bufs=1))
            qt0 = [qpool0.tile([DH + 1, SG], F32R, name=f"qt0_{h}") for h in range(H)]
            wqkp = pre_es.enter_context(tc.tile_pool(name="wqkp", bufs=2))
            qspool = pre_es.enter_context(tc.tile_pool(name="qspool", bufs=2))
            vwp = pre_es.enter_context(tc.tile_pool(name="vwp", bufs=1))
            kp0 = pre_es.enter_context(tc.tile_pool(name="kp0", bufs=2))

            vwb_cache = {}

            def v_proj(ci, half, ts, on_act):
                """V projection for a 256-wide dv quarter, key tiles ts.
                The bias rides as a K=1 ones-row matmul so the psum->SBUF
                move is a plain copy, placeable on ACT (preamble) or DVE."""
                HW = CK // 2
                lo = ci * CK + half * HW
                w = min(D, lo + HW) - lo
                if w <= 0:
                    return
                key = (ci, half)
                if key not in vwb_cache:
                    wvb = vwp.tile([P, KT, HW], F32R, tag="wv",
                                   name=f"wvb{ci}_{half}", bufs=2)
                    nc.sync.dma_start(
                        out=wvb[:], in_=wv.ap()[ci][:, :, half * HW : (half + 1) * HW])
                    vwb_cache[key] = wvb
                wvb = vwb_cache[key]
                h0 = lo // DH
                nh = w // DH
                for t in ts:
                    pt = g.psum_mm.tile([P, HW], F32, tag="mm", name="vpt")
                    for k in range(KT):
                        nc.tensor.matmul(pt[:, :w],
                                         z_full[k][:, t * P : (t + 1) * P],
                                         wvb[:, k, :w],
                                         start=(k == 0), stop=False)
                    nc.tensor.matmul(pt[:, :w], g.ones_row_r[:],
                                     bv_row[0:1, lo : lo + w],
                                     start=False, stop=True)
                    src = pt[:, :w].rearrange("p (h d) -> p h d", d=DH)
                    dst = v_sb[t][:, h0 : h0 + nh, 0:DH]
                    if on_act:
                        nc.scalar.copy(out=dst, in_=src)
                    else:
                        nc.vector.tensor_copy(out=dst, in_=src)

            # pair-0 K runs inside the LN chunk callback so the first QK can
            # start right after the last z chunk lands
            kaug_p0 = []
            for h in range(2):
                kaug = kp0.tile([DH + 1, N], F32R, name=f"kaug_p0_{h}", tag="kaug")
                nc.sync.dma_start(out=kaug[DH : DH + 1, :], in_=onesd.ap())
                kaug_p0.append(kaug)
                kaugs[h] = kaug
            wkb0 = w_load(wk, 0, "wqk", dt=F32R, pool=wqkp)

            def ln_chunk_cb(c):
                if c == 1:
                    # z chunk 0 is live: queries(seg0) for all heads
                    for jp in range(HP):
                        q_proj_pair(jp, z_full, slice(0, SG),
                                    qt0[2 * jp][0:DH, :], qt0[2 * jp + 1][0:DH, :],
                                    qspool, on_act=True)
                sl = slice(c * CK, (c + 1) * CK)
                pt = g.psum_mm.tile([P, CK], F32, tag="mm", name="kpt")
                for k in range(KT):
                    nc.tensor.matmul(pt[:], wkb0[:, k, :], z_full[k][:, sl],
                                     start=(k == 0), stop=(k == KT - 1))
                nc.vector.tensor_copy(out=kaug_p0[0][0:DH, sl], in_=pt[0:DH, :])
                stg = qspool.tile([P, CK], F32R, tag="kstg", bufs=2)
                nc.vector.tensor_copy(out=stg[DH:P, :], in_=pt[DH:P, :])
                nc.sync.dma_start(out=kaug_p0[1][0:DH, sl], in_=stg[DH:P, :])
                v_proj(0, 0, range(4 * c, 4 * c + 4), True)  # heads 0-3

            with ExitStack() as ln_es:
                xsp = ln_es.enter_context(tc.tile_pool(name="xsp", bufs=2))
                lnw1 = ln_es.enter_context(tc.tile_pool(name="lnw1", bufs=1))
                lnps = ln_es.enter_context(tc.tile_pool(name="lnps", bufs=1,
                                                        space="PSUM"))
                _layernorm_fm(tc, g, stream_loader(xT, xsp), N, "ln1",
                              zpool, F32R, wp=lnw1, rstd_on_act=True,
                              lnps=lnps, on_chunk=ln_chunk_cb, z_tiles=z_full,
                              rb=2)
            g.psum_st = es.enter_context(
                tc.tile_pool(name="psum_st", bufs=2, space="PSUM"))
            g.psum_av = es.enter_context(
                tc.tile_pool(name="psum_av", bufs=2, space="PSUM"))

            for s in range(2):
                nc.sync.dma_start(out=kdram.ap()[s], in_=kaug_p0[s][:])

            kpool_a = pre_es.enter_context(tc.tile_pool(name="kpool_a", bufs=4))
            pexp_a = pre_es.enter_context(tc.tile_pool(name="pexp_a", bufs=4))

            def m_rows(jpn):
                for h in (2 * jpn, 2 * jpn + 1):
                    m_shift(h, qt0[h], 0)
                    nc.sync.dma_start(out=qt0[h][DH : DH + 1, :],
                                      in_=mscratch.ap()[h : h + 1, 0:SG])

            def k_and_m(jpn):
                """K for pair jpn, then its seg-0 shift rows into qt0."""
                for h, kaug in zip((2 * jpn, 2 * jpn + 1), k_proj(jpn, kpool_a)):
                    kaugs[h] = kaug
                m_rows(jpn)

            m_rows(0)

            # ============== segment 0 ==============
            def seg0_filler(jp):
                """K/M for pair jp+1, remaining V quarters, Q(seg1)."""
                fills = []
                if jp + 1 < HP:
                    fills.append(lambda jpn=jp + 1: k_and_m(jpn))
                if jp < 4:
                    ci, half = (0, 1) if jp < 2 else (1, 0)
                    fills.append(lambda ci=ci, half=half, lo=(jp % 2) * 8:
                                 v_proj(ci, half, range(lo, lo + 8), False))
                # Q(seg1) goes straight to DRAM from the psum staging
                # tile; its shift row is computed in segment 1.
                fills.append(lambda: q_proj_pair(
                    jp, z_full, slice(SG, NQ),
                    qdram.ap()[2 * jp], qdram.ap()[2 * jp + 1], qspool))
                return fills

            for jp in range(HP):
                attn_pair(jp, 0, seg0_filler(jp), pexp_a)
        # z_full + qt0 + seg0 kaug/pexp pools die here

        # ============== segment 1 (+ seg-0 FFN as filler) ==============
        with ExitStack() as late_es:
            opool1 = late_es.enter_context(tc.tile_pool(name="opool1", bufs=1))
            o_sb[1] = [opool1.tile([P, SG], BF16, name=f"o1_{j}") for j in range(KT)]
            wop = late_es.enter_context(tc.tile_pool(name="wop", bufs=1))
            wobs = [w_load(wo, j, f"wo{j}", pool=wop) for j in range(KT)]
            xmpool = late_es.enter_context(tc.tile_pool(name="xmpool", bufs=1))
            xmid[0] = [xmpool.tile([P, SG], BF16, name=f"xm0_{j}") for j in range(KT)]
            xmid[1] = [xmpool.tile([P, SG], BF16, name=f"xm1_{j}") for j in range(KT)]

            h1p0 = late_es.enter_context(tc.tile_pool(name="h1p0", bufs=1))
            wfp = late_es.enter_context(tc.tile_pool(name="wfp", bufs=2))
            with ExitStack() as s1_es:
                z2p0 = s1_es.enter_context(tc.tile_pool(name="z2p0", bufs=1))
                qpool1 = s1_es.enter_context(tc.tile_pool(name="qpool1", bufs=4))
                kpool_b = s1_es.enter_context(tc.tile_pool(name="kpool_b", bufs=4))
                pexp_b = s1_es.enter_context(tc.tile_pool(name="pexp_b", bufs=4))

                qt1s = {}

                def load_pair(jp):
                    """Stream kaug + q back for pair jp, compute seg-1 shift."""
                    for s in range(2):
                        h = 2 * jp + s
                        kaug = kpool_b.tile([DH + 1, N], F32R,
                                            name=f"kaug1_{h}", tag="kaug")
                        nc.sync.dma_start(out=kaug[:], in_=kdram.ap()[h])
                        kaugs[h] = kaug
                        qt = qpool1.tile([DH + 1, SG], F32R, tag="qt1")
                        nc.sync.dma_start(out=qt[0:DH, :], in_=qdram.ap()[h])
                        m_shift(h, qt, 1)
                        nc.sync.dma_start(out=qt[DH : DH + 1, :],
                                          in_=mscratch.ap()[h : h + 1, SG:NQ])
                        qt1s[h] = qt

                load_pair(0)

                # fillers: prefetches, then out-proj(seg0) + LN2(seg0) + the
                # FFN1(seg0) matmuls (gelu deferred to the tail so ACT stays
                # on the Exp table throughout the attention stream)
                ffn0_pre, ffn0_gelu, ffn0_2 = ffn_fills(0, z2p0, h1p0, wfp, True)
                fills_all = out_proj_fills(0, wfp) + ffn0_pre
                per = (len(fills_all) + HP - 1) // HP
                for jp in range(HP):
                    fills = []
                    if jp + 1 < HP:
                        fills.append(lambda jpn=jp + 1: load_pair(jpn))
                    fills += fills_all[jp * per : (jp + 1) * per]
                    attn_pair(jp, 1, fills, pexp_b)

            # ====== tail: gelu(seg0) + FFN2(seg0) + out-proj/FFN(seg1) ======
            with ExitStack() as s2_es:
                z2p1 = s2_es.enter_context(tc.tile_pool(name="z2p1", bufs=1))
                h1p1 = s2_es.enter_context(tc.tile_pool(name="h1p1", bufs=1))
                ffn1_pre, ffn1_gelu, ffn1_2 = ffn_fills(1, z2p1, h1p1, wfp, True,
                                                        rstd_on_act=True)
                for f in out_proj_fills(1, wfp) + ffn1_pre:
                    f()
                # arm the gelu gate once the final attention output lands
                nc.vector.tensor_scalar_mul(out=g.gelu_gate[:],
                                            in0=o_sb[1][KT - 1][:, SG - 1 : SG],
                                            scalar1=0.0)
                for f in ffn0_gelu + ffn1_gelu + ffn0_2 + ffn1_2:
                    f()


def _tile_w(a, nk, w):
    """[K*128, NOUT] -> [NOUT//w, 128, nk, w] (kernel's stationary-tile order)."""
    kdim = a.shape[0]
    assert kdim == nk * P
    nj = a.shape[1] // w
    out = np.empty((nj, P, nk, w), dtype=a.dtype)
    for j in range(nj):
        blk = a[:, j * w : (j + 1) * w].reshape(nk, P, w)
        out[j] = blk.transpose(1, 0, 2)
    return np.ascontiguousarray(out)


def _tile_x(a, ck=CK):
    """[768, NCOLS] -> [NCOLS//ck, 128, KT, ck]."""
    d, ncols = a.shape
    nc_ = ncols // ck
    out = np.empty((nc_, P, KT, ck), dtype=a.dtype)
    for c in range(nc_):
        blk = a[:, c * ck : (c + 1) * ck].reshape(KT, P, ck)
        out[c] = blk.transpose(1, 0, 2)
    return np.ascontiguousarray(out)


def _prep_inputs(x, ln1_g, ln1_b, w_qkv, b_qkv, w_out, b_out, ln2_g, ln2_b, w1, b1, w2, b2):
    """Host-side prep: fold LN affines into weights, pre-tile, transpose x."""
    import ml_dtypes

    f32, bf = np.float32, ml_dtypes.bfloat16
    ln1_g = np.asarray(ln1_g, f32); ln1_b = np.asarray(ln1_b, f32)
    ln2_g = np.asarray(ln2_g, f32); ln2_b = np.asarray(ln2_b, f32)
    w_qkv = np.asarray(w_qkv, f32); w_out = np.asarray(w_out, f32)
    w1 = np.asarray(w1, f32); w2 = np.asarray(w2, f32)
    b_qkv = np.asarray(b_qkv, f32)

    wq_f = (ln1_g[:, None] * w_qkv[:, 0:D]).astype(f32)
    wk_f = (ln1_g[:, None] * w_qkv[:, D : 2 * D]).astype(f32)
    wv_f = (ln1_g[:, None] * w_qkv[:, 2 * D :]).astype(f32)
    wv_pad = np.zeros((D, 2 * CK), f32)
    wv_pad[:, :D] = wv_f

    common = {
        "wq": _tile_w(wq_f, KT, P),
        "wk": _tile_w(wk_f, KT, P),
        "wv": _tile_x(wv_pad, CK),  # same [c][p][k][ck] layout over dv chunks
        "wo": _tile_w(w_out.astype(bf), KT, P),
        "w1": _tile_w((ln2_g[:, None] * w1).astype(bf), KT, P),
        "w2": _tile_w(w2.astype(bf), MT, P),
        "bq": np.ascontiguousarray(ln1_b @ w_qkv[:, 0:D] + b_qkv[0:D]),
        "bv": np.ascontiguousarray(ln1_b @ w_qkv[:, 2 * D :] + b_qkv[2 * D :]),
        "bo": np.ascontiguousarray(np.asarray(b_out, f32)),
        "b1": np.ascontiguousarray(ln2_b @ w1 + np.asarray(b1, f32)),
        "b2": np.ascontiguousarray(np.asarray(b2, f32)),
        "ident": np.ascontiguousarray(np.eye(P, dtype=bf)),
        "onesd": np.ones((1, N), f32),
    }
    in_maps = []
    for c in range(8):
        b_idx, half = c // 2, c % 2
        xb = np.asarray(x[b_idx], dtype=f32)
        m = dict(common)
        xt = np.ascontiguousarray(xb.T)
        # own tokens first: softmax is invariant to key order, and this lets
        # one compiled program slice its own queries at columns 0:NQ
        xt_perm = np.concatenate(
            [xt[:, half * NQ : (half + 1) * NQ], xt[:, (1 - half) * NQ : (2 - half) * NQ]],
            axis=1)
        m["xT"] = _tile_x(np.ascontiguousarray(xt_perm))
        m["xTq2"] = np.ascontiguousarray(xt_perm[:, :NQ])
        in_maps.append(m)
    return in_maps


_NC_CACHE = {}


def _get_nc():
    if "nc" not in _NC_CACHE:
        _NC_CACHE["nc"] = build_nc()
    return _NC_CACHE["nc"]


def kernel(x, ln1_g, ln1_b, w_qkv, b_qkv, w_out, b_out, ln2_g, ln2_b, w1, b1, w2, b2,
           _trace=False, _tmpdir=None):
    in_maps = _prep_inputs(x, ln1_g, ln1_b, w_qkv, b_qkv, w_out, b_out,
                           ln2_g, ln2_b, w1, b1, w2, b2)
    nc = _get_nc()
    res = run_bass_kernel_spmd(nc, in_maps, list(range(8)), trace=_trace, tmpdir=_tmpdir)
    out = np.empty((B, N, D), dtype=np.float32)
    for c in range(8):
        b_idx, half = c // 2, c % 2
        out[b_idx, half * NQ : (half + 1) * NQ, :] = res.results[c]["yT"].T
    if _trace:
        return out, res
    return out
